# revision 1
# baseline (speedup 1.0000x reference)
"""Trainium2 Bass kernel: multi-head attention (dense transformer block).

Sharding: 8 cores = 4 batches x 2 head-groups (8 heads each).
Each core computes, for its (batch, head-group):
    QT = (Wq_hg^T @ x_b^T) * scale        [512, 2048]   (features on partitions)
    KT = Wk_hg^T @ x_b^T                  [512, 2048]
    V  = x_b @ Wv_hg                      [2048, 512]   (+ ones column per head)
    per head h: PT = exp(KT_h^T @ QT_h)   [2048 j, 2048 i]  (no max-subtract; scores ~N(0,1))
                [O'; Z] = [V_h | 1]^T @ PT                  (unnormalized out + softmax denom)
                OT_h = O' * (1/Z broadcast)                 [64, 2048]
    partial = OT^T @ Wo_hg                [2048, 1024]
Host: out[b] = partial[2b] + partial[2b+1] + b_out.

All matmuls use float32r (FP22 multiply, FP32 accumulate) - full PE speed at
moving dim 512 with ~1e-4 relative error.
"""

import os
import numpy as np

os.environ.setdefault("MYCRO_LOCAL_CACHE", "1")

DIM = 1024
HEADS = 16
DIM_HEAD = 64
INNER = HEADS * DIM_HEAD      # 1024
SEQ = 2048
BATCH = 4
NCORES = 8
HG = 2                        # tensor-parallel head groups
HG_HEADS = HEADS // HG        # 8 heads per core
HG_F = HG_HEADS * DIM_HEAD    # 512 local inner features
SCALE = DIM_HEAD ** -0.5

DCS = DIM // 128              # 8 contraction chunks for QKV
NTS = SEQ // 512              # 4 n-tiles
FBS = HG_F // 128             # 4 feature blocks for QT/KT (2 heads each)
JBS = SEQ // 128              # 16 key blocks
ITS = SEQ // 512              # 4 query tiles
EBS = DIM // 512              # 2 output column tiles

_STATE = None


def _build_module():
    from contextlib import ExitStack, nullcontext
    import concourse.bacc as bacc
    import concourse.tile as tile
    import concourse.mybir as mybir

    f32 = mybir.dt.float32
    f32r = mybir.dt.float32r
    Exp = mybir.ActivationFunctionType.Exp

    nc = bacc.Bacc("TRN2", target_bir_lowering=False, debug=False,
                   num_devices=NCORES)

    xt_d = nc.dram_tensor("xt", [DIM, SEQ], f32r, kind="ExternalInput").ap()
    wq_d = nc.dram_tensor("wq", [DIM, HG_F], f32r, kind="ExternalInput").ap()
    wk_d = nc.dram_tensor("wk", [DIM, HG_F], f32r, kind="ExternalInput").ap()
    wv_d = nc.dram_tensor("wv", [DIM, HG_F], f32r, kind="ExternalInput").ap()
    wo_d = nc.dram_tensor("wo", [HG_F, DIM], f32r, kind="ExternalInput").ap()
    out_d = nc.dram_tensor("out", [SEQ, DIM], f32, kind="ExternalOutput").ap()

    with tile.TileContext(nc) as tc, ExitStack() as ctx:
        pt_ps_pool = ctx.enter_context(
            tc.tile_pool(name="ptps", bufs=2, space="PSUM"))
        acc_ps_pool = ctx.enter_context(
            tc.tile_pool(name="accps", bufs=2, space="PSUM"))
        qt_pool = ctx.enter_context(tc.tile_pool(name="qtp", bufs=FBS))
        kt_pool = ctx.enter_context(tc.tile_pool(name="ktp", bufs=FBS))
        v_pool = ctx.enter_context(tc.tile_pool(name="vpp", bufs=JBS))
        pts_pool = ctx.enter_context(tc.tile_pool(name="ptsb", bufs=2))
        ot_pool = ctx.enter_context(tc.tile_pool(name="otp", bufs=FBS))
        sm_pool = ctx.enter_context(tc.tile_pool(name="smp", bufs=1))

        qt = [qt_pool.tile([128, SEQ], f32r, tag="qt", name=f"qt{i}")
              for i in range(FBS)]
        kt = [kt_pool.tile([128, SEQ], f32r, tag="kt", name=f"kt{i}")
              for i in range(FBS)]
        vp = [v_pool.tile([128, HG_HEADS, DIM_HEAD + 1], f32r, tag="vp",
                          name=f"vp{i}") for i in range(JBS)]

        with tc.tile_pool(name="xtp", bufs=DCS) as xt_pool, \
             tc.tile_pool(name="wvp", bufs=DCS) as wv_pool, \
             tc.tile_pool(name="wqkp", bufs=16) as wqk_pool:

            wvt = []
            for dc in range(DCS):
                t = wv_pool.tile([128, HG_F], f32r, tag="wv", name=f"wv{dc}")
                nc.sync.dma_start(t[:, :], wv_d[dc * 128:(dc + 1) * 128, :])
                wvt.append(t)
            xt = []
            for dc in range(DCS):
                t = xt_pool.tile([128, SEQ], f32r, tag="xt", name=f"xt{dc}")
                nc.sync.dma_start(t[:, :], xt_d[dc * 128:(dc + 1) * 128, :])
                xt.append(t)

            for nb in range(JBS):
                ps = acc_ps_pool.tile([128, HG_HEADS, DIM_HEAD], f32,
                                      tag="acc")
                for dc in range(DCS):
                    nc.tensor.matmul(
                        ps[:, :, :],
                        xt[dc][:, nb * 128:(nb + 1) * 128],
                        wvt[dc][:, :],
                        start=(dc == 0), stop=(dc == DCS - 1))
                nc.vector.tensor_scalar(
                    vp[nb][:, :, DIM_HEAD:DIM_HEAD + 1],
                    ps[:, :, 0:1],
                    0.0, 1.0,
                    mybir.AluOpType.mult, mybir.AluOpType.add)
                nc.vector.tensor_copy(vp[nb][:, :, 0:DIM_HEAD], ps[:, :, :])

            for fb in range(FBS):
                hpq = tc.high_priority() if fb == 0 else nullcontext()
                with hpq:
                  for (w_d, dst, scale, wtag) in (
                          (wq_d, qt, SCALE, "q"), (wk_d, kt, 1.0, "k")):
                      ws = []
                      for dc in range(DCS):
                          t = wqk_pool.tile([128, 128], f32r, tag="wqk",
                                            name=f"w{wtag}{fb}_{dc}")
                          nc.sync.dma_start(
                              t[:, :],
                              w_d[dc * 128:(dc + 1) * 128,
                                  fb * 128:(fb + 1) * 128])
                          ws.append(t)
                      for nt in range(NTS):
                          ps = acc_ps_pool.tile([128, 512], f32, tag="acc")
                          for dc in range(DCS):
                              nc.tensor.matmul(
                                  ps[:, :],
                                  ws[dc][:, :],
                                  xt[dc][:, nt * 512:(nt + 1) * 512],
                                  start=(dc == 0), stop=(dc == DCS - 1))
                          if scale != 1.0:
                              nc.vector.tensor_scalar_mul(
                                  dst[fb][:, nt * 512:(nt + 1) * 512],
                                  ps[:, :], scale)
                          else:
                              nc.vector.tensor_copy(
                                  dst[fb][:, nt * 512:(nt + 1) * 512], ps[:, :])

        with tc.tile_pool(name="wop", bufs=FBS) as wo_pool, \
             tc.tile_pool(name="outs", bufs=4) as out_pool:
            wo = []
            for fc in range(FBS):
                t = wo_pool.tile([128, DIM], f32r, tag="wo", name=f"wo{fc}")
                nc.sync.dma_start(t[:, :], wo_d[fc * 128:(fc + 1) * 128, :])
                wo.append(t)

            def emit_proj(it, ots):
                i0 = it * 512
                for eb in range(EBS):
                    for nb in range(4):
                        ps = acc_ps_pool.tile([128, 512], f32, tag="acc")
                        for fc in range(FBS):
                            nc.tensor.matmul(
                                ps[:, :],
                                ots[fc][:, nb * 128:(nb + 1) * 128],
                                wo[fc][:, eb * 512:(eb + 1) * 512],
                                start=(fc == 0), stop=(fc == FBS - 1))
                        ob = out_pool.tile([128, 512], f32, tag="ob")
                        nc.vector.tensor_copy(ob[:, :], ps[:, :])
                        nc.sync.dma_start(
                            out_d[i0 + nb * 128:i0 + (nb + 1) * 128,
                                  eb * 512:(eb + 1) * 512],
                            ob[:, :])

            ot_hist = {}
            for it in range(ITS):
                i0 = it * 512
                ot = [ot_pool.tile([128, 512], f32r, tag="ot",
                                   name=f"ot{it}_{i}") for i in range(FBS)]
                ot_hist[it] = ot
                hp = tc.high_priority() if it == 0 else nullcontext()
                with hp:
                    for tl in range(FBS):
                        accs = [acc_ps_pool.tile(
                            [DIM_HEAD + 1, 512], f32, tag="att", bufs=2,
                            name=f"acco{it}_{tl}_{k}") for k in range(2)]
                        for jb in range(JBS):
                            ptps = pt_ps_pool.tile([128, 2, 512], f32,
                                                   tag="pt")
                            for k in range(2):
                                p0 = k * 64
                                nc.tensor.matmul(
                                    ptps[:, k:k + 1, :],
                                    kt[tl][p0:p0 + 64,
                                           jb * 128:(jb + 1) * 128],
                                    qt[tl][p0:p0 + 64, i0:i0 + 512],
                                    start=True, stop=True)
                            pts = pts_pool.tile([128, 2, 512], f32r,
                                                tag="pts")
                            nc.scalar.activation(pts[:, :, :], ptps[:, :, :],
                                                 Exp)
                            for k in range(2):
                                h = 2 * tl + k
                                nc.tensor.matmul(
                                    accs[k][:, :],
                                    vp[jb][:, h:h + 1, :],
                                    pts[:, k:k + 1, :],
                                    start=(jb == 0), stop=(jb == JBS - 1))
                        if tl == 0 and it > 0:
                            emit_proj(it - 1, ot_hist[it - 1])
                        for k in range(2):
                            p0 = k * 64
                            acco = accs[k]
                            recip = sm_pool.tile([1, 512], f32r, tag="recip")
                            with nc.allow_low_precision("softmax recip"):
                                nc.vector.reciprocal(
                                    recip[:, :],
                                    acco[DIM_HEAD:DIM_HEAD + 1, :])
                            bc = sm_pool.tile([DIM_HEAD, 512], f32r,
                                              tag="bc")
                            nc.gpsimd.partition_broadcast(bc[:, :],
                                                          recip[:, :])
                            nc.vector.tensor_mul(
                                ot[tl][p0:p0 + 64, :],
                                acco[0:DIM_HEAD, :], bc[:, :])
            emit_proj(ITS - 1, ot_hist[ITS - 1])

    nc.compile()
    return nc


def _make_in_maps(x, w_qkv):
    xt = np.ascontiguousarray(np.asarray(x, np.float32).transpose(0, 2, 1))
    w_qkv = np.asarray(w_qkv, np.float32)
    in_maps = []
    for c in range(NCORES):
        b, hg = divmod(c, HG)
        f0 = hg * HG_F
        in_maps.append({
            "xt": xt[b],
            "wq": np.ascontiguousarray(w_qkv[:, f0:f0 + HG_F]),
            "wk": np.ascontiguousarray(w_qkv[:, INNER + f0:INNER + f0 + HG_F]),
            "wv": np.ascontiguousarray(
                w_qkv[:, 2 * INNER + f0:2 * INNER + f0 + HG_F]),
            "wo": None,  # filled below
        })
    return in_maps


def run(x, w_qkv, w_out, **spmd_kwargs):
    """Build (once) + execute on 8 cores; returns BassKernelResults."""
    global _STATE
    from concourse.bass_utils import run_bass_kernel_spmd
    if _STATE is None:
        _STATE = _build_module()
    w_out = np.asarray(w_out, np.float32)
    in_maps = _make_in_maps(x, w_qkv)
    for c in range(NCORES):
        hg = c % HG
        in_maps[c]["wo"] = np.ascontiguousarray(
            w_out[hg * HG_F:(hg + 1) * HG_F, :])
    return run_bass_kernel_spmd(_STATE, in_maps,
                                core_ids=list(range(NCORES)), **spmd_kwargs)


def kernel(x, w_qkv, w_out, b_out):
    res = run(x, w_qkv, w_out)
    parts = [np.asarray(res.results[c]["out"]) for c in range(NCORES)]
    b_out = np.asarray(b_out, np.float32)
    out = np.stack([parts[HG * b] + parts[HG * b + 1] for b in range(BATCH)])
    out += b_out[None, None, :]
    return out.astype(np.float32, copy=False)

